# revision 24
# baseline (speedup 1.0000x reference)
"""Trainium2 Bass kernel for nn_PairwiseConv (gnn_message_passing).

Reference computation, for each edge e=(i,j) of a sparse adjacency:
    pair[b,o,e] = sum_c W[o,c,0]*x[b,c,i] + W[o,c,1]*x[b,c,j] + bias[o]
    y[b,o,n]    = (sum_{e: i_e=n} pair[b,o,e]) / max(deg_j[n],1)
    y[b,127,n]  = deg_j[n]            (counts channel)
where deg_j[n] = #{e: j_e = n}.

Algebraic reformulation (exact), with r[n] = 1/max(deg_j[n],1) and
a[n] = deg_i[n]*r[n]:
    y[b,o,n] = W1^T U[b,:,n] + W0^T (x[b,:,n]*a[n]) + bias[o]*a[n]
    U[b,c,n] = sum_m x[b,c,m] * AT'[m,n]
    AT'[m,n] = #{e: j_e=m, i_e=n} * r[n]
so the irregular gather/scatter becomes one dense [128,4096]x[4096,512]
matmul per (batch, node-slice) against an fp8 count matrix, followed by
a small weight application per batch.

The big contraction runs in fp8 (e4m3) with DoubleRow perf mode (two
128-row k-tiles per instruction). AT' is built ON DEVICE by GPSIMD
local_scatter from host-packed tables: adjacent fp8 column pairs are
packed into int16 words (local_scatter requires 2-byte dtypes), and the
fp8 matmul view aliases the same SBUF bytes via AP bitcast. This keeps
2 MB of mostly-zero matrix off the DMA queues; the scatter also
zero-fills, so no memset is needed. The weight application + final add
stay bf16/f32; the counts channel is written exactly from a
host-computed f32 degree row.

Sharding: 8 cores = 8 slices of 512 output nodes; each core computes all
4 batches for its slice. Per-core inputs differ only in data; the SPMD
program is identical (scatter table widths are padded to the global
max).
"""

import numpy as np
import ml_dtypes

import concourse.bass as bass
import concourse.mybir as mybir
import concourse.tile as tile
from concourse import bacc, library_config
from concourse.bass_utils import run_bass_kernel_spmd

B = 4
C = 128   # in channels
O = 128   # out channels incl. counts row
N = 4096
SLICE = 512
NCORES = 8
MC = N // 128   # 32 k-chunks of the source-node axis
NG = 8          # scatter groups, 4 chunks each
F32 = mybir.dt.float32
BF16 = mybir.dt.bfloat16
FP8 = mybir.dt.float8e4
I16 = mybir.dt.int16
BF16_NP = ml_dtypes.bfloat16
FP8_NP = ml_dtypes.float8_e4m3
DR = mybir.MatmulPerfMode.DoubleRow
HALF = SLICE // 2


def _pack_scatter(cnt, ni=None):
    """Pack AT' [4096, 512] f32 into per-(group,partition) int16 scatter
    tables. Adjacent fp8 column pairs form one int16 word; group g covers
    source-node chunks [4g, 4g+4) = rows [512g, 512g+512).

    Returns (idx [128, NG*ni] int16, val [128, NG*ni] int16, ni).
    """
    cnt8 = np.ascontiguousarray(cnt.astype(FP8_NP)).view(np.uint8)  # [4096,512]
    pack = cnt8[:, 0::2].astype(np.uint16) | (
        cnt8[:, 1::2].astype(np.uint16) << 8)                       # [4096,256]
    m_idx, t_idx = np.nonzero(pack)
    g = m_idx // 512
    p = m_idx % 128
    mcl = (m_idx // 128) % 4
    elem = (mcl * 256 + t_idx).astype(np.int64)   # [0, 1024) within group
    vals = pack[m_idx, t_idx].astype(np.uint16).view(np.int16)
    cell = g * 128 + p
    order = np.lexsort((elem, cell))
    cell, elem, vals = cell[order], elem[order], vals[order]
    percell = np.bincount(cell, minlength=NG * 128)
    ni_min = int(percell.max()) if len(cell) else 2
    if ni is None:
        ni = ni_min
        ni += ni % 2
        ni = max(ni, 2)
    else:
        assert ni >= ni_min
    idx = np.full((NG * 128, ni), -1, np.int16)
    val = np.zeros((NG * 128, ni), np.int16)
    pos = np.arange(len(cell)) - np.concatenate(([0], np.cumsum(percell)))[cell]
    idx[cell, pos] = elem.astype(np.int16)
    val[cell, pos] = vals
    # [NG*128, ni] -> [128, NG*ni]
    idx = idx.reshape(NG, 128, ni).transpose(1, 0, 2).reshape(128, NG * ni)
    val = val.reshape(NG, 128, ni).transpose(1, 0, 2).reshape(128, NG * ni)
    return np.ascontiguousarray(idx), np.ascontiguousarray(val), ni


def prep_inputs(x, W, b, idx_i, idx_j):
    """Per-core input dicts + scatter table width. Irregular work is host-side."""
    x = np.asarray(x, np.float32)
    W = np.asarray(W, np.float32)
    bias = np.asarray(b, np.float32)
    ii = np.asarray(idx_i).astype(np.int64)
    jj = np.asarray(idx_j).astype(np.int64)

    degj = np.bincount(jj, minlength=N).astype(np.float32)
    degi = np.bincount(ii, minlength=N).astype(np.float32)
    recip = 1.0 / np.maximum(degj, 1.0)

    w01 = np.zeros((128, 2, 128), BF16_NP)
    w01[:, 0, :127] = W[:, :, 0].T
    w01[:, 1, :127] = W[:, :, 1].T
    brow = np.zeros((1, 128), BF16_NP)
    brow[0, :127] = bias

    # xtp[b][p, mc, c] = x[b, c, 128*mc + p]   (fp8, shared across cores)
    xtp = [
        np.ascontiguousarray(
            x[bi].T.reshape(MC, 128, C).transpose(1, 0, 2)
        ).astype(FP8_NP)
        for bi in range(B)
    ]

    percore = []
    ni = 2
    for s in range(NCORES):
        base = s * SLICE
        sl = slice(base, base + SLICE)
        a = degi[sl] * recip[sl]
        sel = (ii >= base) & (ii < base + SLICE)
        key = jj[sel] * SLICE + (ii[sel] - base)
        cnt = np.bincount(key, minlength=N * SLICE).astype(np.float32)
        cnt = cnt.reshape(N, SLICE) * recip[sl][None, :]
        _, _, ni_s = _pack_scatter(cnt)
        ni = max(ni, ni_s)
        percore.append((sl, a, cnt))

    in_maps = []
    for s in range(NCORES):
        sl, a, cnt = percore[s]
        idxT, valT, _ = _pack_scatter(cnt, ni=ni)
        xs = np.ascontiguousarray(
            np.stack([x[bi, :, sl] * a[None, :] for bi in range(B)], axis=1)
        ).astype(BF16_NP)
        bar = np.zeros((1, 128 + SLICE), BF16_NP)
        bar[0, :128] = brow[0]
        bar[0, 128:] = a.astype(BF16_NP)
        degj4 = np.tile(degj[sl].astype(BF16_NP)[None, :], (1, B))
        m = {
            "w01": w01,
            "bar": bar,
            "degj4": np.ascontiguousarray(degj4),
            "idxT": idxT,
            "valT": valT,
            "xs": xs,
        }
        for bi in range(B):
            m[f"xtp{bi}"] = xtp[bi]
        in_maps.append(m)
    return in_maps, ni


def build_program(ni):
    nc = bacc.Bacc("TRN2", target_bir_lowering=False, debug=False,
                   num_devices=NCORES)

    xtps = [nc.dram_tensor(f"xtp{bi}", [128, MC, 128], FP8,
                           kind="ExternalInput") for bi in range(B)]
    idxT_d = nc.dram_tensor("idxT", [128, NG * ni], I16, kind="ExternalInput")
    valT_d = nc.dram_tensor("valT", [128, NG * ni], I16, kind="ExternalInput")
    xs_d = nc.dram_tensor("xs", [128, B, SLICE], BF16, kind="ExternalInput")
    w01_d = nc.dram_tensor("w01", [128, 2, 128], BF16, kind="ExternalInput")
    bar_d = nc.dram_tensor("bar", [1, 128 + SLICE], BF16, kind="ExternalInput")
    degj4_d = nc.dram_tensor("degj4", [1, B * SLICE], BF16,
                             kind="ExternalInput")
    youts = [nc.dram_tensor(f"y{bi}", [O, SLICE], BF16, kind="ExternalOutput")
             for bi in range(B)]

    with tile.TileContext(nc) as tc:
        with (
            tc.tile_pool(name="const", bufs=1) as constp,
            tc.tile_pool(name="tab", bufs=1) as tabp,
            tc.tile_pool(name="at", bufs=1) as atp,
            tc.tile_pool(name="xp", bufs=1) as xp,
            tc.tile_pool(name="xu", bufs=1) as xup,
            tc.tile_pool(name="ost", bufs=1) as ostp,
            tc.tile_pool(name="ps_u", bufs=1, space="PSUM") as ps_u,
            tc.tile_pool(name="ps_2", bufs=2, space="PSUM") as ps_2,
        ):
            at_t = atp.tile([128, MC, SLICE], FP8)
            xtp_t = [xp.tile([128, MC, 128], FP8, tag=f"xtp{bi}",
                             name=f"xtp{bi}") for bi in range(B)]
            xu_t = xup.tile([128, B, 2, SLICE], BF16)
            ost_t = ostp.tile([O, B, SLICE], BF16)
            w01_t = constp.tile([128, 2, 128], BF16)
            bar_t = constp.tile([1, 128 + SLICE], BF16)
            idx_t = tabp.tile([128, NG * ni], I16)
            val_t = tabp.tile([128, NG * ni], I16)
            brow_t = bar_t[:, 0:128]
            arow_t = bar_t[:, 128:]

            # ---- preload the GPSIMD local_scatter ucode library so the
            # ~2.5us lib switch overlaps the framework preamble + table DMA
            nc.gpsimd.load_library(library_config.local_scatter)

            # ---- input DMAs: tables first, then xtp in 16-chunk pieces
            # (2 KB per partition row = full DMA efficiency) round-robin
            # across batches so no batch's chunk-k arrives much later than
            # another's (the in-order PE stalls on the straggler batch)
            nc.sync.dma_start(idx_t[:], idxT_d[:])
            nc.scalar.dma_start(val_t[:], valT_d[:])
            for lo, hi in ((0, 16), (16, MC)):
                nc.sync.dma_start(xtp_t[0][:, lo:hi, :], xtps[0][:, lo:hi, :])
                nc.scalar.dma_start(xtp_t[2][:, lo:hi, :],
                                    xtps[2][:, lo:hi, :])
                nc.sync.dma_start(xtp_t[1][:, lo:hi, :], xtps[1][:, lo:hi, :])
                nc.scalar.dma_start(xtp_t[3][:, lo:hi, :],
                                    xtps[3][:, lo:hi, :])
            nc.sync.dma_start(w01_t[:], w01_d[:])
            nc.scalar.dma_start(bar_t[:], bar_d[:])
            nc.scalar.dma_start(xu_t[:, :, 0, :], xs_d[:, :, :])
            nc.scalar.dma_start(ost_t[127:128, :, :], degj4_d[:])

            # ---- AT' build: 8 GPSIMD scatters of 4 chunks each ----
            for g in range(NG):
                nc.gpsimd.local_scatter(
                    out_ap=at_t[:, 4 * g:4 * g + 4, :].bitcast(I16),
                    data_ap=val_t[:, g * ni:(g + 1) * ni],
                    idxs_ap=idx_t[:, g * ni:(g + 1) * ni],
                    channels=128, num_elems=1024, num_idxs=ni,
                )

            # ---- U_b = x_b @ AT' : fp8 DoubleRow, K=256 per matmul ----
            u_ps = [ps_u.tile([128, SLICE], F32, tag=f"u{bi}",
                              name=f"u{bi}") for bi in range(B)]

            def pair_mm(k, bi):
                nc.tensor.matmul(
                    u_ps[bi][:, :],
                    xtp_t[bi][:, 2 * k:2 * k + 2, :],
                    at_t[:, 2 * k:2 * k + 2, :],
                    start=(k == 0), stop=(k == MC // 2 - 1),
                    perf_mode=DR, skip_group_check=True,
                )

            def epilogue(bi):
                if bi % 2 == 0:
                    cast_f = nc.scalar.copy
                    copy_f = nc.vector.tensor_copy
                else:
                    cast_f = nc.vector.tensor_copy
                    copy_f = nc.scalar.copy
                ps2 = ps_2.tile([128, SLICE], F32, tag="p2", name=f"ps2_{bi}")
                for h in range(2):
                    hs = slice(h * HALF, (h + 1) * HALF)
                    cast_f(xu_t[:, bi, 1, hs], u_ps[bi][:, hs])
                nc.tensor.matmul(ps2[:, :], w01_t[:, 0, :],
                                 xu_t[:, bi, 0, :],
                                 start=True, stop=False, skip_group_check=True)
                nc.tensor.matmul(ps2[:, :], w01_t[:, 1, :],
                                 xu_t[:, bi, 1, :],
                                 start=False, stop=False, skip_group_check=True)
                nc.tensor.matmul(ps2[:, :], brow_t[:, :], arow_t[:, :],
                                 start=False, stop=True, skip_group_check=True)
                for h in range(2):
                    hs = slice(h * HALF, (h + 1) * HALF)
                    copy_f(ost_t[0:127, bi, hs], ps2[0:127, hs])
                eng = nc.sync if bi % 2 == 0 else nc.scalar
                eng.dma_start(youts[bi][:, :], ost_t[:, bi, :])

            # Phased schedule matched to real DMA arrival (while all 8 cores
            # hammer HBM, only the sync queue's early pieces — xtp batches
            # 0,1 — have landed). The tile scheduler reorders by its own
            # optimistic DMA model, so each phase is pinned with a virtual
            # wait_until floor; runtime waits still come from semaphores.
            # Phase A: batches 0,1 stream first (doubles as PE p-state
            # warmup); their epilogues sandwich between batch 2's and batch
            # 3's streams so outputs hide under pair matmuls.
            for k in range(10):
                for bi in (0, 1):
                    pair_mm(k, bi)
            with tc.tile_wait_until(0.02):
                for bi in (0, 1):
                    for k in range(10, MC // 2):
                        pair_mm(k, bi)
            with tc.tile_wait_until(0.04):
                for k in range(MC // 2):
                    pair_mm(k, 2)
            with tc.tile_wait_until(0.06):
                epilogue(0)
                epilogue(1)
            with tc.tile_wait_until(0.08):
                for k in range(MC // 2):
                    pair_mm(k, 3)
            with tc.tile_wait_until(0.10):
                epilogue(2)
            with tc.tile_wait_until(0.12):
                epilogue(3)

    nc.compile()
    return nc


def kernel(x, W, b, idx_i, idx_j):
    in_maps, ni = prep_inputs(x, W, b, idx_i, idx_j)
    nc = build_program(ni)
    res = run_bass_kernel_spmd(nc, in_maps, list(range(NCORES)))
    y = np.empty((B, O, N), np.float32)
    for s in range(NCORES):
        for bi in range(B):
            y[bi, :, s * SLICE:(s + 1) * SLICE] = res.results[s][
                f"y{bi}"].astype(np.float32)
    return y


if __name__ == "__main__":
    rng = np.random.default_rng(0)
    x = rng.standard_normal((B, C, N), np.float32)
    W = rng.standard_normal((127, C, 2), np.float32) * 0.05
    b = rng.standard_normal((127,), np.float32) * 0.05
    idx_i = rng.integers(0, N, 131072)
    idx_j = rng.integers(0, N, 131072)
    y = kernel(x, W, b, idx_i, idx_j)
    print("ok", y.shape, float(np.abs(y).mean()))


# revision 25
# speedup vs baseline: 1.2267x; 1.2267x over previous
"""Trainium2 Bass kernel for nn_PairwiseConv (gnn_message_passing).

Reference computation, for each edge e=(i,j) of a sparse adjacency:
    pair[b,o,e] = sum_c W[o,c,0]*x[b,c,i] + W[o,c,1]*x[b,c,j] + bias[o]
    y[b,o,n]    = (sum_{e: i_e=n} pair[b,o,e]) / max(deg_j[n],1)
    y[b,127,n]  = deg_j[n]            (counts channel)
where deg_j[n] = #{e: j_e = n}.

Algebraic reformulation (exact), with r[n] = 1/max(deg_j[n],1) and
a[n] = deg_i[n]*r[n]:
    y[b,o,n] = W1^T U[b,:,n] + W0^T (x[b,:,n]*a[n]) + bias[o]*a[n]
    U[b,c,n] = sum_m x[b,c,m] * AT'[m,n]
    AT'[m,n] = #{e: j_e=m, i_e=n} * r[n]
so the irregular gather/scatter becomes one dense [128,4096]x[4096,512]
matmul per (batch, node-slice) against an fp8 count matrix, followed by
a small weight application per batch.

The big contraction runs in fp8 (e4m3) with DoubleRow perf mode (two
128-row k-tiles per instruction). AT' is built ON DEVICE by GPSIMD
local_scatter from host-packed tables: adjacent fp8 column pairs are
packed into int16 words (local_scatter requires 2-byte dtypes), and the
fp8 matmul view aliases the same SBUF bytes via AP bitcast. This keeps
2 MB of mostly-zero matrix off the DMA queues; the scatter also
zero-fills, so no memset is needed. The weight application + final add
stay bf16/f32; the counts channel is written exactly from a
host-computed f32 degree row.

Sharding: 8 cores = 8 slices of 512 output nodes; each core computes all
4 batches for its slice. Per-core inputs differ only in data; the SPMD
program is identical (scatter table widths are padded to the global
max).
"""

import numpy as np
import ml_dtypes

import concourse.bass as bass
import concourse.mybir as mybir
import concourse.tile as tile
from concourse import bacc, library_config
from concourse.bass_utils import run_bass_kernel_spmd

B = 4
C = 128   # in channels
O = 128   # out channels incl. counts row
N = 4096
SLICE = 512
NCORES = 8
MC = N // 128   # 32 k-chunks of the source-node axis
NG = 8          # scatter groups, 4 chunks each
F32 = mybir.dt.float32
BF16 = mybir.dt.bfloat16
FP8 = mybir.dt.float8e4
I16 = mybir.dt.int16
BF16_NP = ml_dtypes.bfloat16
FP8_NP = ml_dtypes.float8_e4m3
DR = mybir.MatmulPerfMode.DoubleRow
HALF = SLICE // 2


def _pack_scatter(cnt, ni=None):
    """Pack AT' [4096, 512] f32 into per-(group,partition) int16 scatter
    tables. Adjacent fp8 column pairs form one int16 word; group g covers
    source-node chunks [4g, 4g+4) = rows [512g, 512g+512).

    Returns (idx [128, NG*ni] int16, val [128, NG*ni] int16, ni).
    """
    cnt8 = np.ascontiguousarray(cnt.astype(FP8_NP)).view(np.uint8)  # [4096,512]
    pack = cnt8[:, 0::2].astype(np.uint16) | (
        cnt8[:, 1::2].astype(np.uint16) << 8)                       # [4096,256]
    m_idx, t_idx = np.nonzero(pack)
    g = m_idx // 512
    p = m_idx % 128
    mcl = (m_idx // 128) % 4
    elem = (mcl * 256 + t_idx).astype(np.int64)   # [0, 1024) within group
    vals = pack[m_idx, t_idx].astype(np.uint16).view(np.int16)
    cell = g * 128 + p
    order = np.lexsort((elem, cell))
    cell, elem, vals = cell[order], elem[order], vals[order]
    percell = np.bincount(cell, minlength=NG * 128)
    ni_min = int(percell.max()) if len(cell) else 2
    if ni is None:
        ni = ni_min
        ni += ni % 2
        ni = max(ni, 2)
    else:
        assert ni >= ni_min
    idx = np.full((NG * 128, ni), -1, np.int16)
    val = np.zeros((NG * 128, ni), np.int16)
    pos = np.arange(len(cell)) - np.concatenate(([0], np.cumsum(percell)))[cell]
    idx[cell, pos] = elem.astype(np.int16)
    val[cell, pos] = vals
    # [NG*128, ni] -> [128, NG*ni]
    idx = idx.reshape(NG, 128, ni).transpose(1, 0, 2).reshape(128, NG * ni)
    val = val.reshape(NG, 128, ni).transpose(1, 0, 2).reshape(128, NG * ni)
    return np.ascontiguousarray(idx), np.ascontiguousarray(val), ni


def prep_inputs(x, W, b, idx_i, idx_j):
    """Per-core input dicts + scatter table width. Irregular work is host-side."""
    x = np.asarray(x, np.float32)
    W = np.asarray(W, np.float32)
    bias = np.asarray(b, np.float32)
    ii = np.asarray(idx_i).astype(np.int64)
    jj = np.asarray(idx_j).astype(np.int64)

    degj = np.bincount(jj, minlength=N).astype(np.float32)
    degi = np.bincount(ii, minlength=N).astype(np.float32)
    recip = 1.0 / np.maximum(degj, 1.0)

    w01 = np.zeros((128, 2, 128), BF16_NP)
    w01[:, 0, :127] = W[:, :, 0].T
    w01[:, 1, :127] = W[:, :, 1].T
    brow = np.zeros((1, 128), BF16_NP)
    brow[0, :127] = bias

    # xtp[b][p, mc, c] = x[b, c, 128*mc + p]   (fp8, shared across cores)
    xtp = [
        np.ascontiguousarray(
            x[bi].T.reshape(MC, 128, C).transpose(1, 0, 2)
        ).astype(FP8_NP)
        for bi in range(B)
    ]

    percore = []
    ni = 2
    for s in range(NCORES):
        base = s * SLICE
        sl = slice(base, base + SLICE)
        a = degi[sl] * recip[sl]
        sel = (ii >= base) & (ii < base + SLICE)
        key = jj[sel] * SLICE + (ii[sel] - base)
        cnt = np.bincount(key, minlength=N * SLICE).astype(np.float32)
        cnt = cnt.reshape(N, SLICE) * recip[sl][None, :]
        _, _, ni_s = _pack_scatter(cnt)
        ni = max(ni, ni_s)
        percore.append((sl, a, cnt))

    in_maps = []
    for s in range(NCORES):
        sl, a, cnt = percore[s]
        idxT, valT, _ = _pack_scatter(cnt, ni=ni)
        xs = np.ascontiguousarray(
            np.stack([x[bi, :, sl] * a[None, :] for bi in range(B)], axis=1)
        ).astype(BF16_NP)
        bar = np.zeros((1, 128 + SLICE), BF16_NP)
        bar[0, :128] = brow[0]
        bar[0, 128:] = a.astype(BF16_NP)
        degj4 = np.tile(degj[sl].astype(BF16_NP)[None, :], (1, B))
        m = {
            "w01": w01,
            "bar": bar,
            "degj4": np.ascontiguousarray(degj4),
            "idxT": idxT,
            "valT": valT,
            "xs": xs,
        }
        for bi in range(B):
            m[f"xtp{bi}"] = xtp[bi]
        in_maps.append(m)
    return in_maps, ni


def build_program(ni):
    nc = bacc.Bacc("TRN2", target_bir_lowering=False, debug=False,
                   num_devices=NCORES)

    xtps = [nc.dram_tensor(f"xtp{bi}", [128, MC, 128], FP8,
                           kind="ExternalInput") for bi in range(B)]
    idxT_d = nc.dram_tensor("idxT", [128, NG * ni], I16, kind="ExternalInput")
    valT_d = nc.dram_tensor("valT", [128, NG * ni], I16, kind="ExternalInput")
    xs_d = nc.dram_tensor("xs", [128, B, SLICE], BF16, kind="ExternalInput")
    w01_d = nc.dram_tensor("w01", [128, 2, 128], BF16, kind="ExternalInput")
    bar_d = nc.dram_tensor("bar", [1, 128 + SLICE], BF16, kind="ExternalInput")
    degj4_d = nc.dram_tensor("degj4", [1, B * SLICE], BF16,
                             kind="ExternalInput")
    youts = [nc.dram_tensor(f"y{bi}", [O, SLICE], BF16, kind="ExternalOutput")
             for bi in range(B)]

    with tile.TileContext(nc) as tc:
        with (
            tc.tile_pool(name="const", bufs=1) as constp,
            tc.tile_pool(name="tab", bufs=1) as tabp,
            tc.tile_pool(name="at", bufs=1) as atp,
            tc.tile_pool(name="xp", bufs=1) as xp,
            tc.tile_pool(name="xu", bufs=1) as xup,
            tc.tile_pool(name="ost", bufs=1) as ostp,
            tc.tile_pool(name="ps_u", bufs=1, space="PSUM") as ps_u,
            tc.tile_pool(name="ps_2", bufs=2, space="PSUM") as ps_2,
        ):
            at_t = atp.tile([128, MC, SLICE], FP8)
            xtp_t = [xp.tile([128, MC, 128], FP8, tag=f"xtp{bi}",
                             name=f"xtp{bi}") for bi in range(B)]
            xu_t = xup.tile([128, B, 2, SLICE], BF16)
            ost_t = ostp.tile([O, B, SLICE], BF16)
            w01_t = constp.tile([128, 2, 128], BF16)
            bar_t = constp.tile([1, 128 + SLICE], BF16)
            idx_t = tabp.tile([128, NG * ni], I16)
            val_t = tabp.tile([128, NG * ni], I16)
            brow_t = bar_t[:, 0:128]
            arow_t = bar_t[:, 128:]

            # ---- preload the GPSIMD local_scatter ucode library so the
            # ~2.5us lib switch overlaps the framework preamble + table DMA
            nc.gpsimd.load_library(library_config.local_scatter)

            # ---- input DMAs: tables first, then xtp pieces round-robin
            # across batches so no batch's chunk-k arrives much later than
            # another's (the in-order PE stalls on the straggler batch)
            nc.sync.dma_start(idx_t[:], idxT_d[:])
            nc.scalar.dma_start(val_t[:], valT_d[:])
            for lo, hi in ((0, 8), (8, 20), (20, MC)):
                nc.sync.dma_start(xtp_t[0][:, lo:hi, :], xtps[0][:, lo:hi, :])
                nc.scalar.dma_start(xtp_t[2][:, lo:hi, :],
                                    xtps[2][:, lo:hi, :])
                nc.sync.dma_start(xtp_t[1][:, lo:hi, :], xtps[1][:, lo:hi, :])
                nc.scalar.dma_start(xtp_t[3][:, lo:hi, :],
                                    xtps[3][:, lo:hi, :])
            nc.sync.dma_start(w01_t[:], w01_d[:])
            nc.scalar.dma_start(bar_t[:], bar_d[:])
            nc.scalar.dma_start(xu_t[:, :, 0, :], xs_d[:, :, :])
            nc.scalar.dma_start(ost_t[127:128, :, :], degj4_d[:])

            # ---- AT' build: 8 GPSIMD scatters of 4 chunks each ----
            for g in range(NG):
                nc.gpsimd.local_scatter(
                    out_ap=at_t[:, 4 * g:4 * g + 4, :].bitcast(I16),
                    data_ap=val_t[:, g * ni:(g + 1) * ni],
                    idxs_ap=idx_t[:, g * ni:(g + 1) * ni],
                    channels=128, num_elems=1024, num_idxs=ni,
                )

            # ---- U_b = x_b @ AT' : fp8 DoubleRow, K=256 per matmul ----
            u_ps = [ps_u.tile([128, SLICE], F32, tag=f"u{bi}",
                              name=f"u{bi}") for bi in range(B)]

            def pair_mm(k, bi):
                nc.tensor.matmul(
                    u_ps[bi][:, :],
                    xtp_t[bi][:, 2 * k:2 * k + 2, :],
                    at_t[:, 2 * k:2 * k + 2, :],
                    start=(k == 0), stop=(k == MC // 2 - 1),
                    perf_mode=DR, skip_group_check=True,
                )

            # deep staggered batch tails: batch b's accumulation finishes
            # ~TAIL pairs before batch b+1's, so earlier batches' epilogues
            # and output DMAs hide under the remaining pair matmuls
            TAIL = 10
            for k in range(MC // 2 - TAIL):
                for bi in range(B):
                    pair_mm(k, bi)
            for bi in range(B):
                for k in range(MC // 2 - TAIL, MC // 2):
                    pair_mm(k, bi)

            # ---- per-batch epilogue ----
            # casts/copies alternate scalar/vector; output DMAs alternate
            # sync/scalar, with scalar's issued last on that queue so a
            # waiting DMA issue never blocks a cast/copy behind it.
            ydmas = []
            for bi in range(B):
                if bi % 2 == 0:
                    cast_f = nc.scalar.copy
                    copy_f = nc.vector.tensor_copy
                else:
                    cast_f = nc.vector.tensor_copy
                    copy_f = nc.scalar.copy
                ps2 = ps_2.tile([128, SLICE], F32, tag="p2", name=f"ps2_{bi}")
                for h in range(2):
                    hs = slice(h * HALF, (h + 1) * HALF)
                    cast_f(xu_t[:, bi, 1, hs], u_ps[bi][:, hs])
                nc.tensor.matmul(ps2[:, :], w01_t[:, 0, :],
                                 xu_t[:, bi, 0, :],
                                 start=True, stop=False, skip_group_check=True)
                nc.tensor.matmul(ps2[:, :], w01_t[:, 1, :],
                                 xu_t[:, bi, 1, :],
                                 start=False, stop=False, skip_group_check=True)
                nc.tensor.matmul(ps2[:, :], brow_t[:, :], arow_t[:, :],
                                 start=False, stop=True, skip_group_check=True)
                for h in range(2):
                    hs = slice(h * HALF, (h + 1) * HALF)
                    copy_f(ost_t[0:127, bi, hs], ps2[0:127, hs])
                if bi % 2 == 0:
                    nc.sync.dma_start(youts[bi][:, :], ost_t[:, bi, :])
                else:
                    ydmas.append(bi)
            for bi in ydmas:
                nc.scalar.dma_start(youts[bi][:, :], ost_t[:, bi, :])

    nc.compile()
    return nc


def kernel(x, W, b, idx_i, idx_j):
    in_maps, ni = prep_inputs(x, W, b, idx_i, idx_j)
    nc = build_program(ni)
    res = run_bass_kernel_spmd(nc, in_maps, list(range(NCORES)))
    y = np.empty((B, O, N), np.float32)
    for s in range(NCORES):
        for bi in range(B):
            y[bi, :, s * SLICE:(s + 1) * SLICE] = res.results[s][
                f"y{bi}"].astype(np.float32)
    return y


if __name__ == "__main__":
    rng = np.random.default_rng(0)
    x = rng.standard_normal((B, C, N), np.float32)
    W = rng.standard_normal((127, C, 2), np.float32) * 0.05
    b = rng.standard_normal((127,), np.float32) * 0.05
    idx_i = rng.integers(0, N, 131072)
    idx_j = rng.integers(0, N, 131072)
    y = kernel(x, W, b, idx_i, idx_j)
    print("ok", y.shape, float(np.abs(y).mean()))
